# revision 1
# baseline (speedup 1.0000x reference)
"""Trainium2 Bass kernel for nn_CrossAttention (seq_len==1 cross attention,
dual-stream transformer block pair).

Math notes (exact simplifications, valid for any input values):
  - Both attentions have seq_len==1 for q and kv, so softmax over the single
    kv position is exactly 1.0 and attention output == V projection:
        mha(q_in, kv_in) = (kv_in @ wv.T + bv) @ out_w.T + out_b
    The q/k projections are dead code.  Folding the two matmuls:
        attn = kv_in @ (out_w @ wv).T + (out_w @ bv + out_b)
  - LayerNorm affine (g, b) of ln1/ln2 is folded into the following FFN
    weights host-side; residual-path affine and biases are applied on-device
    only when they are non-trivial (they are zeros/ones for the reference
    setup_inputs, so the fast path emits no extra instructions).

v3 layout: host converts dna/mol to bf16 (transpose-DMA reads the input DRAM
tensor directly).  Macro tile R=512 rows.  DMA issue is spread over the three
DMA-capable queues (gpsimd: straight x + out, sync: xbar transposes, scalar:
startup weights) with startup ordering chosen so tile 0 is never starved
behind the 5 MB weight preload.  The per-tile PE stream is software-pipelined
one tile deep — FFN2 of tile t-1 runs between attn and zT/FFN1 of tile t —
so the PE never waits on the LayerNorm chain and the HAM clock gate stays
warm.  LN normalize runs on ACT (Lrelu with alpha=1 as identity, per-partition
scale=1/std, bias=-mean/std); stats stay on DVE.  Output is bf16, widened to
fp32 on the host.
"""

import numpy as np
import ml_dtypes
from contextlib import ExitStack

import concourse.bass as bass
import concourse.tile as tile
from concourse import bacc, mybir
from concourse.bass_utils import run_bass_kernel_spmd

E = 512
HID = 1024
NCORES = 8
EPS = 1e-5
P = 128

BF16 = mybir.dt.bfloat16
F32 = mybir.dt.float32
BF = ml_dtypes.bfloat16

_prog_cache = {}


def _build_program(rows_per_core: int, rmacro: int, flags: tuple):
    """Build + compile the per-core Bass program.

    flags = (use_c0, use_c1, use_d0, use_d1, use_e0, use_e1,
             aff_a0, aff_a1, aff_b0, aff_b1)
    """
    (use_c0, use_c1, use_d0, use_d1, use_e0, use_e1,
     aff_a0, aff_a1, aff_b0, aff_b1) = flags
    use_c = (use_c0, use_c1)
    use_d = (use_d0, use_d1)
    use_e = (use_e0, use_e1)
    aff_a = (aff_a0, aff_a1)
    aff_b = (aff_b0, aff_b1)

    R = rmacro
    NT = rows_per_core // R
    RC = R // P
    KE = E // P    # 4 K-chunks over E
    KH = HID // P  # 8 K-chunks over HID

    nc = bacc.Bacc("TRN2", target_bir_lowering=False, debug=False,
                   num_devices=NCORES)

    dna = nc.dram_tensor("dna", [NT, R, E], BF16, kind="ExternalInput").ap()
    mol = nc.dram_tensor("mol", [NT, R, E], BF16, kind="ExternalInput").ap()
    out = nc.dram_tensor("out", [NT, R, 2 * E], BF16,
                         kind="ExternalOutput").ap()

    wts = {}
    for s in range(2):
        wts[f"w{s}"] = nc.dram_tensor(f"w{s}", [P, KE, E], BF16,
                                      kind="ExternalInput").ap()
        wts[f"u{s}"] = nc.dram_tensor(f"u{s}", [P, KE, HID], BF16,
                                      kind="ExternalInput").ap()
        wts[f"v{s}"] = nc.dram_tensor(f"v{s}", [P, KH, E], BF16,
                                      kind="ExternalInput").ap()
        if use_c[s]:
            wts[f"c{s}"] = nc.dram_tensor(f"c{s}", [1, E], BF16,
                                          kind="ExternalInput").ap()
        if use_d[s]:
            wts[f"d{s}"] = nc.dram_tensor(f"d{s}", [1, HID], BF16,
                                          kind="ExternalInput").ap()
        if use_e[s]:
            wts[f"e{s}"] = nc.dram_tensor(f"e{s}", [1, E], BF16,
                                          kind="ExternalInput").ap()
        if aff_a[s]:
            wts[f"ga{s}"] = nc.dram_tensor(f"ga{s}", [1, E], BF16,
                                           kind="ExternalInput").ap()
            wts[f"ba{s}"] = nc.dram_tensor(f"ba{s}", [1, E], BF16,
                                           kind="ExternalInput").ap()
        if aff_b[s]:
            wts[f"gb{s}"] = nc.dram_tensor(f"gb{s}", [1, E], F32,
                                           kind="ExternalInput").ap()
            wts[f"bb{s}"] = nc.dram_tensor(f"bb{s}", [1, E], F32,
                                           kind="ExternalInput").ap()

    SUB = mybir.AluOpType.subtract
    MULT = mybir.AluOpType.mult
    Relu = mybir.ActivationFunctionType.Relu
    Sqrt = mybir.ActivationFunctionType.Sqrt
    Lrelu = mybir.ActivationFunctionType.Lrelu

    with tile.TileContext(nc) as tc:
        with ExitStack() as ctx:
            const = ctx.enter_context(tc.tile_pool(name="const", bufs=1))
            xin = ctx.enter_context(tc.tile_pool(name="xin", bufs=4))
            xtp = ctx.enter_context(tc.tile_pool(name="xtp", bufs=4))
            hpre = ctx.enter_context(tc.tile_pool(name="hpre", bufs=10))
            zpool = ctx.enter_context(tc.tile_pool(name="zpool", bufs=6))
            ztp = ctx.enter_context(tc.tile_pool(name="ztp", bufs=4))
            gpool = ctx.enter_context(tc.tile_pool(name="gpool", bufs=4))
            ypool = ctx.enter_context(tc.tile_pool(name="ypool", bufs=10))
            opool = ctx.enter_context(tc.tile_pool(name="opool", bufs=16))
            stats = ctx.enter_context(tc.tile_pool(name="stats", bufs=40))
            psA = ctx.enter_context(
                tc.tile_pool(name="psA", bufs=4, space="PSUM"))
            psG = ctx.enter_context(
                tc.tile_pool(name="psG", bufs=2, space="PSUM"))
            psT = ctx.enter_context(
                tc.tile_pool(name="psT", bufs=2, space="PSUM"))

            # --- const tiles (no DMA yet; issue order is set below) ---
            w_sb = {}
            for name, ap in wts.items():
                t = const.tile(list(ap.shape), ap.dtype, tag=f"w_{name}")
                w_sb[name] = t

            def loads(mt):
                """issue straight + transposed loads of both streams.

                gpsimd: straight x (dna first: LN1 s0 consumes it first).
                sync: xbar transposes (mol first: attn s0 needs molT).
                """
                xt_d = xin.tile([P, RC, E], BF16, tag="xin")
                xt_m = xin.tile([P, RC, E], BF16, tag="xin")
                for xt, src in ((xt_d, dna), (xt_m, mol)):
                    for rc in range(RC):
                        nc.gpsimd.dma_start(
                            out=xt[:, rc, :],
                            in_=src[mt, rc * P:(rc + 1) * P, :])
                xT_d = xtp.tile([P, KE, R], BF16, tag="xT")
                xT_m = xtp.tile([P, KE, R], BF16, tag="xT")
                for xT, src in ((xT_m, mol), (xT_d, dna)):
                    for c in range(KE):
                        nc.sync.dma_start_transpose(
                            out=xT[:, c, :],
                            in_=src[mt, :, c * P:(c + 1) * P])
                return [xt_d, xt_m], [xT_d, xT_m]

            # --- startup issue schedule (per-queue ~85 GB/s, ~12us/MB) ---
            # Weight-need times (PE): w0@5 w1@9 u0@17 u1@24 v0@28 v1@35.
            # Big weights are split in halves across queues so each lands
            # just in time without delaying tile-0/1 x loads or transposes.
            nc.scalar.dma_start(out=w_sb["w0"][...], in_=wts["w0"])
            nc.scalar.dma_start(out=w_sb["w1"][...], in_=wts["w1"])
            x_s, xT_s = loads(0)       # gpsimd: dna0,mol0; sync: transposes
            nc.scalar.dma_start(out=w_sb["u0"][:, :2, :],
                                in_=wts["u0"][:, :2, :])
            nc.sync.dma_start(out=w_sb["u0"][:, 2:, :],
                              in_=wts["u0"][:, 2:, :])
            nc.gpsimd.dma_start(out=w_sb["u1"][:, 2:, :],
                                in_=wts["u1"][:, 2:, :])
            nc.sync.dma_start(out=w_sb["u1"][:, :2, :],
                              in_=wts["u1"][:, :2, :])
            nc.scalar.dma_start(out=w_sb["v0"][...], in_=wts["v0"])
            nc.gpsimd.dma_start(out=w_sb["v1"][...], in_=wts["v1"])
            for name in wts:
                if name[0] not in "wuv":
                    nc.scalar.dma_start(out=w_sb[name][...], in_=wts[name])

            ident = const.tile([P, P], BF16, tag="ident")
            from concourse.masks import make_identity
            make_identity(nc, ident[...])
            # replicated affine tiles (only when needed)
            rep = {}
            for s in range(2):
                if aff_a[s]:
                    for nm in (f"ga{s}", f"ba{s}"):
                        r = const.tile([P, E], BF16, tag=f"rep_{nm}")
                        nc.sync.dma_start(out=r[...],
                                          in_=wts[nm].to_broadcast((P, E)))
                        rep[nm] = r
                if aff_b[s]:
                    for nm in (f"gb{s}", f"bb{s}"):
                        r = const.tile([P, E], F32, tag=f"rep_{nm}")
                        nc.sync.dma_start(out=r[...],
                                          in_=wts[nm].to_broadcast((P, E)))
                        rep[nm] = r

            eps_sb = const.tile([P, 1], F32, tag="eps")
            nc.vector.memset(eps_sb[...], EPS)
            ones_sb = const.tile([1, R], BF16, tag="ones")
            nc.vector.memset(ones_sb[...], 1.0)

            def layernorm(dst, src):
                """normalize src [P, E] into dst: (src - m) / std."""
                st6 = stats.tile([P, 6], F32, tag="st6")
                nc.vector.bn_stats(out=st6[...], in_=src)
                mv = stats.tile([P, 2], F32, tag="mv")
                nc.vector.bn_aggr(out=mv[...], in_=st6[...])
                inv = stats.tile([P, 1], F32, tag="inv")
                nc.scalar.activation(out=inv[...], in_=mv[:, 1:2], func=Sqrt,
                                     bias=eps_sb[...], scale=1.0)
                nc.vector.reciprocal(out=inv[...], in_=inv[...])
                nc.vector.tensor_scalar(
                    out=dst, in0=src, scalar1=mv[:, 0:1],
                    scalar2=inv[...], op0=SUB, op1=MULT)

            def ln_group(srcs, dsts):
                """batched LN over RC chunks: one sqrt+recip for the group."""
                mv = stats.tile([P, RC, 2], F32, tag="mv")
                for rc in range(RC):
                    st6 = stats.tile([P, 6], F32, tag="st6")
                    nc.vector.bn_stats(out=st6[...], in_=srcs[rc])
                    nc.vector.bn_aggr(out=mv[:, rc, :], in_=st6[...])
                inv = stats.tile([P, RC, 1], F32, tag="inv")
                nc.scalar.activation(out=inv[...], in_=mv[:, :, 1:2],
                                     func=Sqrt, bias=eps_sb[...], scale=1.0)
                nc.vector.reciprocal(out=inv[...], in_=inv[...])
                for rc in range(RC):
                    nc.vector.tensor_scalar(
                        out=dsts[rc], in0=srcs[rc],
                        scalar1=mv[:, rc, 0:1], scalar2=inv[:, rc, 0:1],
                        op0=SUB, op1=MULT)

            def attn_ln1(s, x_s, xT_s):
                """attn + LN1 -> z (bf16), h1 (residual input for LN2)."""
                x = x_s[s]
                kvT = xT_s[1 - s]
                z = zpool.tile([P, RC, E], BF16, tag="z")
                hps = []
                for rc in range(RC):
                    ps = psA.tile([P, E], F32, tag="psA")
                    for c in range(KE):
                        nc.tensor.matmul(
                            ps[...],
                            kvT[:, c, rc * P:(rc + 1) * P],
                            w_sb[f"w{s}"][:, c, :],
                            start=(c == 0),
                            stop=(c == KE - 1 and not use_c[s]))
                    if use_c[s]:
                        nc.tensor.matmul(ps[...], ones_sb[:, 0:P],
                                         w_sb[f"c{s}"][...],
                                         start=False, stop=True)
                    hp = hpre.tile([P, E], BF16, tag="hpre")
                    nc.vector.tensor_add(hp[...], ps[...], x[:, rc, :])
                    layernorm(z[:, rc, :], hp[...])
                    hps.append(hp)
                if aff_a[s]:
                    h1 = zpool.tile([P, RC, E], BF16, tag="h1")
                    for rc in range(RC):
                        nc.vector.tensor_mul(h1[:, rc, :], z[:, rc, :],
                                             rep[f"ga{s}"][...])
                        nc.vector.tensor_add(h1[:, rc, :], h1[:, rc, :],
                                             rep[f"ba{s}"][...])
                    return z, h1
                return z, z

            def zt_ffn1(s, z):
                """zT via PE transpose, then FFN1 + relu -> gT (bf16)."""
                zT = ztp.tile([P, KE, R], BF16, tag="zT")
                for c in range(KE):
                    pt = psT.tile([P, R], BF16, tag="psT")
                    for rc in range(RC):
                        nc.tensor.transpose(
                            pt[:, rc * P:(rc + 1) * P],
                            z[:, rc, c * P:(c + 1) * P],
                            ident[...])
                    nc.scalar.copy(out=zT[:, c, :], in_=pt[...])
                gt = gpool.tile([P, KH, R], BF16, tag="gt")
                for j in range(KH):
                    pg = psG.tile([P, R], F32, tag="psG")
                    for c in range(KE):
                        nc.tensor.matmul(
                            pg[...],
                            w_sb[f"u{s}"][:, c, j * P:(j + 1) * P],
                            zT[:, c, :],
                            start=(c == 0),
                            stop=(c == KE - 1 and not use_d[s]))
                    if use_d[s]:
                        nc.tensor.matmul(
                            pg[...], w_sb[f"d{s}"][:, j * P:(j + 1) * P],
                            ones_sb[...], start=False, stop=True)
                    nc.scalar.activation(out=gt[:, j, :], in_=pg[...],
                                         func=Relu)
                return gt

            def ffn2_ln2(s, mt, gt, h1):
                """FFN2 + LN2 -> out DMA for stream s."""
                ys = []
                for rc in range(RC):
                    pf = psA.tile([P, E], F32, tag="psA")
                    for j in range(KH):
                        nc.tensor.matmul(
                            pf[...],
                            gt[:, j, rc * P:(rc + 1) * P],
                            w_sb[f"v{s}"][:, j, :],
                            start=(j == 0),
                            stop=(j == KH - 1 and not use_e[s]))
                    if use_e[s]:
                        nc.tensor.matmul(pf[...], ones_sb[:, 0:P],
                                         w_sb[f"e{s}"][...],
                                         start=False, stop=True)
                    y = ypool.tile([P, E], BF16, tag="y")
                    nc.vector.tensor_add(y[...], pf[...], h1[:, rc, :])
                    o = opool.tile([P, E], BF16, tag="ob")
                    if aff_b[s]:
                        of = opool.tile([P, E], F32, tag="of")
                        layernorm(of[...], y[...])
                        nc.vector.tensor_mul(of[...], of[...],
                                             rep[f"gb{s}"][...])
                        nc.vector.tensor_add(o[...], of[...],
                                             rep[f"bb{s}"][...])
                    else:
                        layernorm(o[...], y[...])
                    nc.gpsimd.dma_start(
                        out=out[mt, rc * P:(rc + 1) * P, s * E:(s + 1) * E],
                        in_=o[...])

            def pe_warm(n):
                """dummy N=128 matmuls: keep the HAM clock gate at K=8/8
                while startup DMAs starve the PE of real work."""
                wm = psA.tile([P, E], F32, tag="psA")
                for _ in range(n):
                    nc.tensor.matmul(wm[:, 0:P], ident[...], ident[...],
                                     start=True, stop=True)

            # --- main loop, FFN2 software-pipelined one tile deep ---
            pe_warm(96)
            pend = None
            for mt in range(NT):
                warm = 24 if mt < 3 else 0
                z0, h10 = attn_ln1(0, x_s, xT_s)
                if warm:
                    pe_warm(warm)
                z1, h11 = attn_ln1(1, x_s, xT_s)
                if warm:
                    pe_warm(warm)
                if pend is not None:
                    pmt, pg0, ph0, pg1, ph1 = pend
                    ffn2_ln2(0, pmt, pg0, ph0)
                    ffn2_ln2(1, pmt, pg1, ph1)
                if mt + 1 < NT:
                    x_s, xT_s = loads(mt + 1)
                gt0 = zt_ffn1(0, z0)
                if warm:
                    pe_warm(warm)
                gt1 = zt_ffn1(1, z1)
                if warm:
                    pe_warm(warm)
                pend = (mt, gt0, h10, gt1, h11)
            pmt, pg0, ph0, pg1, ph1 = pend
            ffn2_ln2(0, pmt, pg0, ph0)
            ffn2_ln2(1, pmt, pg1, ph1)

    nc.compile()
    return nc


def _prep_host(inputs):
    """Fold weights host-side; returns (inputs, weight arrays, flags)."""
    g = {k: np.asarray(v, dtype=np.float32) for k, v in inputs.items()}

    def trivial(a, val):
        return bool(np.all(a == val))

    def kchunks(a, nk, dt=BF):
        # [K, N] -> [P, nk, N] (chunk c = rows c*P:(c+1)*P)
        k, n = a.shape
        assert k == nk * P
        return np.ascontiguousarray(
            a.reshape(nk, P, n).transpose(1, 0, 2)).astype(dt)

    arrs = {}
    flags = []
    for s, (aw, ab, ow, ob, lna_g, lna_b, lnb_g, lnb_b, w1, b1, w2, b2) in \
            enumerate((
                (g["a1_in_w"], g["a1_in_b"], g["a1_out_w"], g["a1_out_b"],
                 g["ln1_g"], g["ln1_b"], g["ln3_g"], g["ln3_b"],
                 g["f1_w1"], g["f1_b1"], g["f1_w2"], g["f1_b2"]),
                (g["a2_in_w"], g["a2_in_b"], g["a2_out_w"], g["a2_out_b"],
                 g["ln2_g"], g["ln2_b"], g["ln4_g"], g["ln4_b"],
                 g["f2_w1"], g["f2_b1"], g["f2_w2"], g["f2_b2"]))):
        wv = aw[2 * E:3 * E]
        bv = ab[2 * E:3 * E]
        W = ow @ wv                      # [E, E]; attn = kv @ W.T + c
        c = ow @ bv + ob                 # [E]
        U = w1 * lna_g[None, :]          # LN1 gain folded into FFN1
        d = b1 + w1 @ lna_b              # LN1 bias folded into FFN1 bias
        V = w2                           # [E, HID]
        e = b2                           # [E]
        arrs[f"w{s}"] = kchunks(W.T, E // P)
        arrs[f"u{s}"] = kchunks(U.T, E // P)
        arrs[f"v{s}"] = kchunks(V.T, HID // P)
        uc = not trivial(c, 0.0)
        ud = not trivial(d, 0.0)
        ue = not trivial(e, 0.0)
        fa = not (trivial(lna_g, 1.0) and trivial(lna_b, 0.0))
        fb = not (trivial(lnb_g, 1.0) and trivial(lnb_b, 0.0))
        if uc:
            arrs[f"c{s}"] = c.reshape(1, E).astype(BF)
        if ud:
            arrs[f"d{s}"] = d.reshape(1, HID).astype(BF)
        if ue:
            arrs[f"e{s}"] = e.reshape(1, E).astype(BF)
        if fa:
            arrs[f"ga{s}"] = lna_g.reshape(1, E).astype(BF)
            arrs[f"ba{s}"] = lna_b.reshape(1, E).astype(BF)
        if fb:
            arrs[f"gb{s}"] = lnb_g.reshape(1, E).astype(np.float32)
            arrs[f"bb{s}"] = lnb_b.reshape(1, E).astype(np.float32)
        flags.append((uc, ud, ue, fa, fb))

    (uc0, ud0, ue0, fa0, fb0), (uc1, ud1, ue1, fa1, fb1) = flags
    flag_t = (uc0, uc1, ud0, ud1, ue0, ue1, fa0, fa1, fb0, fb1)
    return g, arrs, flag_t


def _pick_rmacro(rows_per_core):
    for r in (512, 256, 128):
        if rows_per_core % r == 0:
            return r
    raise ValueError(f"rows_per_core {rows_per_core} not divisible by 128")


def kernel(**inputs):
    g, arrs, flag_t = _prep_host(inputs)
    B = g["dna"].shape[0]
    rows_per_core = B // NCORES
    rmacro = _pick_rmacro(rows_per_core)
    key = (rows_per_core, rmacro, flag_t)
    if key not in _prog_cache:
        _prog_cache[key] = _build_program(rows_per_core, rmacro, flag_t)
    nc = _prog_cache[key]

    NT = rows_per_core // rmacro
    dna_bf = np.asarray(g["dna"], dtype=BF)
    mol_bf = np.asarray(g["mol"], dtype=BF)
    in_maps = []
    for i in range(NCORES):
        sl = slice(i * rows_per_core, (i + 1) * rows_per_core)
        im = {
            "dna": dna_bf[sl].reshape(NT, rmacro, E),
            "mol": mol_bf[sl].reshape(NT, rmacro, E),
        }
        im.update(arrs)
        in_maps.append(im)

    res = run_bass_kernel_spmd(nc, in_maps, list(range(NCORES)))
    outs = [r["out"].reshape(rows_per_core, 2 * E) for r in res.results]
    return np.concatenate(outs, axis=0).astype(np.float32)



# revision 3
# speedup vs baseline: 1.1740x; 1.1740x over previous
"""Trainium2 Bass kernel for nn_CrossAttention (seq_len==1 cross attention,
dual-stream transformer block pair).

Math notes (exact simplifications, valid for any input values):
  - Both attentions have seq_len==1 for q and kv, so softmax over the single
    kv position is exactly 1.0 and attention output == V projection:
        mha(q_in, kv_in) = (kv_in @ wv.T + bv) @ out_w.T + out_b
    The q/k projections are dead code.  Folding the two matmuls:
        attn = kv_in @ (out_w @ wv).T + (out_w @ bv + out_b)
  - LayerNorm affine (g, b) of ln1/ln2 is folded into the following FFN
    weights host-side; residual-path affine and biases are applied on-device
    only when they are non-trivial (they are zeros/ones for the reference
    setup_inputs, so the fast path emits no extra instructions).
  - LayerNorm is scale invariant: LN(c*x) == LN(x).  This lets per-tensor
    pow2 scales (used by the fp8 path) cancel without any rescale ops.

v4 layout: host pre-transposes dna/mol (and pre-permutes all tile layouts)
so every SBUF tile is filled by ONE straight DMA — no xbar DMA transposes.
Identity/constant tiles are created BEFORE any DMA issue so the PE warmup
stream starts immediately instead of queueing behind the weight preload.
Queues: sync = transposed x, gpsimd = straight x, scalar = weights + out.
Per-tile PE stream is software-pipelined one tile deep (FFN2 of tile t-1
runs between attn and zT/FFN1 of tile t).  LN sqrt/recip batched per group.
"""

import numpy as np
import ml_dtypes
from contextlib import ExitStack

import concourse.bass as bass
import concourse.tile as tile
from concourse import bacc, mybir
from concourse.bass_utils import run_bass_kernel_spmd

E = 512
HID = 1024
NCORES = 8
EPS = 1e-5
P = 128

BF16 = mybir.dt.bfloat16
F32 = mybir.dt.float32
BF = ml_dtypes.bfloat16

_prog_cache = {}


def _build_program(rows_per_core: int, rmacro: int, flags: tuple):
    """Build + compile the per-core Bass program.

    flags = (use_c0, use_c1, use_d0, use_d1, use_e0, use_e1,
             aff_a0, aff_a1, aff_b0, aff_b1)
    """
    (use_c0, use_c1, use_d0, use_d1, use_e0, use_e1,
     aff_a0, aff_a1, aff_b0, aff_b1) = flags
    use_c = (use_c0, use_c1)
    use_d = (use_d0, use_d1)
    use_e = (use_e0, use_e1)
    aff_a = (aff_a0, aff_a1)
    aff_b = (aff_b0, aff_b1)

    R = rmacro
    NT = rows_per_core // R
    RC = R // P
    KE = E // P    # 4 K-chunks over E
    KH = HID // P  # 8 K-chunks over HID

    nc = bacc.Bacc("TRN2", target_bir_lowering=False, debug=False,
                   num_devices=NCORES)

    # straight x: xs[mt, p, rc, e] = x[mt*R + rc*P + p, e]
    # transposed x: xt[mt, p, c, r] = x[mt*R + r, c*P + p]
    xs_d = {}
    xt_d = {}
    for s, nm in ((0, "dna"), (1, "mol")):
        xs_d[s] = nc.dram_tensor(f"xs_{nm}", [NT, P, RC, E], BF16,
                                 kind="ExternalInput").ap()
        xt_d[s] = nc.dram_tensor(f"xt_{nm}", [NT, P, KE, R], BF16,
                                 kind="ExternalInput").ap()
    # out[mt, s, p, rc, e] = result[mt*R + rc*P + p, s*E + e]
    out = nc.dram_tensor("out", [NT, 2, P, RC, E], BF16,
                         kind="ExternalOutput").ap()

    wts = {}
    for s in range(2):
        wts[f"w{s}"] = nc.dram_tensor(f"w{s}", [P, KE, E], BF16,
                                      kind="ExternalInput").ap()
        wts[f"u{s}"] = nc.dram_tensor(f"u{s}", [P, KE, HID], BF16,
                                      kind="ExternalInput").ap()
        wts[f"v{s}"] = nc.dram_tensor(f"v{s}", [P, KH, E], BF16,
                                      kind="ExternalInput").ap()
        if use_c[s]:
            wts[f"c{s}"] = nc.dram_tensor(f"c{s}", [1, E], BF16,
                                          kind="ExternalInput").ap()
        if use_d[s]:
            wts[f"d{s}"] = nc.dram_tensor(f"d{s}", [1, HID], BF16,
                                          kind="ExternalInput").ap()
        if use_e[s]:
            wts[f"e{s}"] = nc.dram_tensor(f"e{s}", [1, E], BF16,
                                          kind="ExternalInput").ap()
        if aff_a[s]:
            wts[f"ga{s}"] = nc.dram_tensor(f"ga{s}", [1, E], BF16,
                                           kind="ExternalInput").ap()
            wts[f"ba{s}"] = nc.dram_tensor(f"ba{s}", [1, E], BF16,
                                           kind="ExternalInput").ap()
        if aff_b[s]:
            wts[f"gb{s}"] = nc.dram_tensor(f"gb{s}", [1, E], F32,
                                           kind="ExternalInput").ap()
            wts[f"bb{s}"] = nc.dram_tensor(f"bb{s}", [1, E], F32,
                                           kind="ExternalInput").ap()

    SUB = mybir.AluOpType.subtract
    MULT = mybir.AluOpType.mult
    Relu = mybir.ActivationFunctionType.Relu
    Sqrt = mybir.ActivationFunctionType.Sqrt

    with tile.TileContext(nc) as tc:
        with ExitStack() as ctx:
            const = ctx.enter_context(tc.tile_pool(name="const", bufs=1))
            xin = ctx.enter_context(tc.tile_pool(name="xin", bufs=4))
            xtp = ctx.enter_context(tc.tile_pool(name="xtp", bufs=4))
            hpre = ctx.enter_context(tc.tile_pool(name="hpre", bufs=10))
            zpool = ctx.enter_context(tc.tile_pool(name="zpool", bufs=6))
            ztp = ctx.enter_context(tc.tile_pool(name="ztp", bufs=4))
            gpool = ctx.enter_context(tc.tile_pool(name="gpool", bufs=4))
            ypool = ctx.enter_context(tc.tile_pool(name="ypool", bufs=10))
            opool = ctx.enter_context(tc.tile_pool(name="opool", bufs=4))
            stats = ctx.enter_context(tc.tile_pool(name="stats", bufs=40))
            psA = ctx.enter_context(
                tc.tile_pool(name="psA", bufs=4, space="PSUM"))
            psG = ctx.enter_context(
                tc.tile_pool(name="psG", bufs=2, space="PSUM"))
            psT = ctx.enter_context(
                tc.tile_pool(name="psT", bufs=2, space="PSUM"))

            # --- consts FIRST: PE warmup must not wait on the DMA preload ---
            ident = const.tile([P, P], BF16, tag="ident")
            from concourse.masks import make_identity
            make_identity(nc, ident[...])
            eps_sb = const.tile([P, 1], F32, tag="eps")
            nc.vector.memset(eps_sb[...], EPS)
            ones_sb = const.tile([1, R], BF16, tag="ones")
            nc.vector.memset(ones_sb[...], 1.0)

            w_sb = {}
            for name, ap in wts.items():
                t = const.tile(list(ap.shape), ap.dtype, tag=f"w_{name}")
                w_sb[name] = t

            def loads(mt):
                """issue straight + transposed loads of both streams.

                sync: transposed x (mol first: attn s0 needs molT).
                gpsimd: straight x (dna first: residual s0 consumes it first).
                """
                xt_m = xtp.tile([P, KE, R], BF16, tag="xT")
                xt_dd = xtp.tile([P, KE, R], BF16, tag="xT")
                nc.sync.dma_start(out=xt_m[...], in_=xt_d[1][mt])
                nc.sync.dma_start(out=xt_dd[...], in_=xt_d[0][mt])
                x_dd = xin.tile([P, RC, E], BF16, tag="xin")
                x_m = xin.tile([P, RC, E], BF16, tag="xin")
                nc.gpsimd.dma_start(out=x_dd[...], in_=xs_d[0][mt])
                nc.gpsimd.dma_start(out=x_m[...], in_=xs_d[1][mt])
                return [x_dd, x_m], [xt_dd, xt_m]

            # --- startup issue schedule (per-queue ~85 GB/s, ~12us/MB) ---
            # PE needs (us, approx): w0@12 w1@15 u0@19 u1@26 v0@48 v1@55.
            nc.scalar.dma_start(out=w_sb["w0"][...], in_=wts["w0"])
            x_s, xT_s = loads(0)
            nc.scalar.dma_start(out=w_sb["w1"][...], in_=wts["w1"])
            nc.scalar.dma_start(out=w_sb["u0"][:, :2, :],
                                in_=wts["u0"][:, :2, :])
            nc.sync.dma_start(out=w_sb["u0"][:, 2:, :],
                              in_=wts["u0"][:, 2:, :])
            nc.gpsimd.dma_start(out=w_sb["u1"][:, 2:, :],
                                in_=wts["u1"][:, 2:, :])
            nc.sync.dma_start(out=w_sb["u1"][:, :2, :],
                              in_=wts["u1"][:, :2, :])
            nc.scalar.dma_start(out=w_sb["v0"][...], in_=wts["v0"])
            nc.gpsimd.dma_start(out=w_sb["v1"][...], in_=wts["v1"])
            for name in wts:
                if name[0] not in "wuv":
                    nc.scalar.dma_start(out=w_sb[name][...], in_=wts[name])

            # replicated affine tiles (only when needed)
            rep = {}
            for s in range(2):
                if aff_a[s]:
                    for nm in (f"ga{s}", f"ba{s}"):
                        r = const.tile([P, E], BF16, tag=f"rep_{nm}")
                        nc.sync.dma_start(out=r[...],
                                          in_=wts[nm].to_broadcast((P, E)))
                        rep[nm] = r
                if aff_b[s]:
                    for nm in (f"gb{s}", f"bb{s}"):
                        r = const.tile([P, E], F32, tag=f"rep_{nm}")
                        nc.sync.dma_start(out=r[...],
                                          in_=wts[nm].to_broadcast((P, E)))
                        rep[nm] = r

            def ln_group(srcs, dsts):
                """batched LN over RC chunks: one sqrt+recip for the group."""
                mv = stats.tile([P, RC, 2], F32, tag="mv")
                for rc in range(RC):
                    st6 = stats.tile([P, 6], F32, tag="st6")
                    nc.vector.bn_stats(out=st6[...], in_=srcs[rc])
                    nc.vector.bn_aggr(out=mv[:, rc, :], in_=st6[...])
                inv = stats.tile([P, RC, 1], F32, tag="inv")
                nc.scalar.activation(out=inv[...], in_=mv[:, :, 1:2],
                                     func=Sqrt, bias=eps_sb[...], scale=1.0)
                nc.vector.reciprocal(out=inv[...], in_=inv[...])
                for rc in range(RC):
                    nc.vector.tensor_scalar(
                        out=dsts[rc], in0=srcs[rc],
                        scalar1=mv[:, rc, 0:1], scalar2=inv[:, rc, 0:1],
                        op0=SUB, op1=MULT)

            def attn_ln1(s, x_s, xT_s):
                """attn + LN1 -> z (bf16), h1 (residual input for LN2)."""
                x = x_s[s]
                kvT = xT_s[1 - s]
                z = zpool.tile([P, RC, E], BF16, tag="z")
                hps = []
                for rc in range(RC):
                    ps = psA.tile([P, E], F32, tag="psA")
                    for c in range(KE):
                        nc.tensor.matmul(
                            ps[...],
                            kvT[:, c, rc * P:(rc + 1) * P],
                            w_sb[f"w{s}"][:, c, :],
                            start=(c == 0),
                            stop=(c == KE - 1 and not use_c[s]))
                    if use_c[s]:
                        nc.tensor.matmul(ps[...], ones_sb[:, 0:P],
                                         w_sb[f"c{s}"][...],
                                         start=False, stop=True)
                    hp = hpre.tile([P, E], BF16, tag="hpre")
                    nc.vector.tensor_add(hp[...], ps[...], x[:, rc, :])
                    hps.append(hp)
                ln_group(hps, [z[:, rc, :] for rc in range(RC)])
                if aff_a[s]:
                    h1 = zpool.tile([P, RC, E], BF16, tag="h1")
                    for rc in range(RC):
                        nc.vector.tensor_mul(h1[:, rc, :], z[:, rc, :],
                                             rep[f"ga{s}"][...])
                        nc.vector.tensor_add(h1[:, rc, :], h1[:, rc, :],
                                             rep[f"ba{s}"][...])
                    return z, h1
                return z, z

            def zt_ffn1(s, z):
                """zT via PE transpose, then FFN1 + relu -> gT (bf16)."""
                zT = ztp.tile([P, KE, R], BF16, tag="zT")
                for c in range(KE):
                    pt = psT.tile([P, R], BF16, tag="psT")
                    for rc in range(RC):
                        nc.tensor.transpose(
                            pt[:, rc * P:(rc + 1) * P],
                            z[:, rc, c * P:(c + 1) * P],
                            ident[...])
                    nc.scalar.copy(out=zT[:, c, :], in_=pt[...])
                gt = gpool.tile([P, KH, R], BF16, tag="gt")
                for j in range(KH):
                    pg = psG.tile([P, R], F32, tag="psG")
                    for c in range(KE):
                        nc.tensor.matmul(
                            pg[...],
                            w_sb[f"u{s}"][:, c, j * P:(j + 1) * P],
                            zT[:, c, :],
                            start=(c == 0),
                            stop=(c == KE - 1 and not use_d[s]))
                    if use_d[s]:
                        nc.tensor.matmul(
                            pg[...], w_sb[f"d{s}"][:, j * P:(j + 1) * P],
                            ones_sb[...], start=False, stop=True)
                    nc.scalar.activation(out=gt[:, j, :], in_=pg[...],
                                         func=Relu)
                return gt

            def ffn2_ln2(s, mt, gt, h1):
                """FFN2 + LN2 -> out DMA for stream s."""
                ys = []
                for rc in range(RC):
                    pf = psA.tile([P, E], F32, tag="psA")
                    for j in range(KH):
                        nc.tensor.matmul(
                            pf[...],
                            gt[:, j, rc * P:(rc + 1) * P],
                            w_sb[f"v{s}"][:, j, :],
                            start=(j == 0),
                            stop=(j == KH - 1 and not use_e[s]))
                    if use_e[s]:
                        nc.tensor.matmul(pf[...], ones_sb[:, 0:P],
                                         w_sb[f"e{s}"][...],
                                         start=False, stop=True)
                    y = ypool.tile([P, E], BF16, tag="y")
                    nc.vector.tensor_add(y[...], pf[...], h1[:, rc, :])
                    ys.append(y)
                o = opool.tile([P, RC, E], BF16, tag="ob")
                if aff_b[s]:
                    ofs = []
                    for rc in range(RC):
                        of = opool.tile([P, E], F32, tag="of")
                        ofs.append(of)
                    ln_group([y[...] for y in ys],
                             [of[...] for of in ofs])
                    for rc in range(RC):
                        nc.vector.tensor_mul(ofs[rc][...], ofs[rc][...],
                                             rep[f"gb{s}"][...])
                        nc.vector.tensor_add(o[:, rc, :], ofs[rc][...],
                                             rep[f"bb{s}"][...])
                else:
                    ln_group([y[...] for y in ys],
                             [o[:, rc, :] for rc in range(RC)])
                # out DMA split in halves: overlaps LN of later chunks
                h = RC // 2 if RC >= 2 else 1
                nc.scalar.dma_start(out=out[mt, s, :, :h, :],
                                    in_=o[:, :h, :])
                if RC >= 2:
                    nc.scalar.dma_start(out=out[mt, s, :, h:, :],
                                        in_=o[:, h:, :])

            def pe_warm(n):
                """dummy N=128 matmuls: keep the PE p-state/clock ramped
                while startup DMAs starve the PE of real work."""
                wm = psA.tile([P, E], F32, tag="psA")
                for _ in range(n):
                    nc.tensor.matmul(wm[:, 0:P], ident[...], ident[...],
                                     start=True, stop=True)

            # --- main loop, FFN2 software-pipelined one tile deep ---
            pe_warm(96)
            pend = None
            for mt in range(NT):
                warm = 24 if mt < 3 else 0
                z0, h10 = attn_ln1(0, x_s, xT_s)
                if warm:
                    pe_warm(warm)
                z1, h11 = attn_ln1(1, x_s, xT_s)
                if warm:
                    pe_warm(warm)
                if pend is not None:
                    pmt, pg0, ph0, pg1, ph1 = pend
                    ffn2_ln2(0, pmt, pg0, ph0)
                    ffn2_ln2(1, pmt, pg1, ph1)
                if mt + 1 < NT:
                    x_s, xT_s = loads(mt + 1)
                gt0 = zt_ffn1(0, z0)
                if warm:
                    pe_warm(warm)
                gt1 = zt_ffn1(1, z1)
                if warm:
                    pe_warm(warm)
                pend = (mt, gt0, h10, gt1, h11)
            pmt, pg0, ph0, pg1, ph1 = pend
            ffn2_ln2(0, pmt, pg0, ph0)
            ffn2_ln2(1, pmt, pg1, ph1)

    nc.compile()
    return nc


def _prep_host(inputs):
    """Fold weights host-side; returns (inputs, weight arrays, flags)."""
    g = {k: np.asarray(v, dtype=np.float32) for k, v in inputs.items()}

    def trivial(a, val):
        return bool(np.all(a == val))

    def kchunks(a, nk, dt=BF):
        # [K, N] -> [P, nk, N] (chunk c = rows c*P:(c+1)*P)
        k, n = a.shape
        assert k == nk * P
        return np.ascontiguousarray(
            a.reshape(nk, P, n).transpose(1, 0, 2)).astype(dt)

    arrs = {}
    flags = []
    for s, (aw, ab, ow, ob, lna_g, lna_b, lnb_g, lnb_b, w1, b1, w2, b2) in \
            enumerate((
                (g["a1_in_w"], g["a1_in_b"], g["a1_out_w"], g["a1_out_b"],
                 g["ln1_g"], g["ln1_b"], g["ln3_g"], g["ln3_b"],
                 g["f1_w1"], g["f1_b1"], g["f1_w2"], g["f1_b2"]),
                (g["a2_in_w"], g["a2_in_b"], g["a2_out_w"], g["a2_out_b"],
                 g["ln2_g"], g["ln2_b"], g["ln4_g"], g["ln4_b"],
                 g["f2_w1"], g["f2_b1"], g["f2_w2"], g["f2_b2"]))):
        wv = aw[2 * E:3 * E]
        bv = ab[2 * E:3 * E]
        W = ow @ wv                      # [E, E]; attn = kv @ W.T + c
        c = ow @ bv + ob                 # [E]
        U = w1 * lna_g[None, :]          # LN1 gain folded into FFN1
        d = b1 + w1 @ lna_b              # LN1 bias folded into FFN1 bias
        V = w2                           # [E, HID]
        e = b2                           # [E]
        arrs[f"w{s}"] = kchunks(W.T, E // P)
        arrs[f"u{s}"] = kchunks(U.T, E // P)
        arrs[f"v{s}"] = kchunks(V.T, HID // P)
        uc = not trivial(c, 0.0)
        ud = not trivial(d, 0.0)
        ue = not trivial(e, 0.0)
        fa = not (trivial(lna_g, 1.0) and trivial(lna_b, 0.0))
        fb = not (trivial(lnb_g, 1.0) and trivial(lnb_b, 0.0))
        if uc:
            arrs[f"c{s}"] = c.reshape(1, E).astype(BF)
        if ud:
            arrs[f"d{s}"] = d.reshape(1, HID).astype(BF)
        if ue:
            arrs[f"e{s}"] = e.reshape(1, E).astype(BF)
        if fa:
            arrs[f"ga{s}"] = lna_g.reshape(1, E).astype(BF)
            arrs[f"ba{s}"] = lna_b.reshape(1, E).astype(BF)
        if fb:
            arrs[f"gb{s}"] = lnb_g.reshape(1, E).astype(np.float32)
            arrs[f"bb{s}"] = lnb_b.reshape(1, E).astype(np.float32)
        flags.append((uc, ud, ue, fa, fb))

    (uc0, ud0, ue0, fa0, fb0), (uc1, ud1, ue1, fa1, fb1) = flags
    flag_t = (uc0, uc1, ud0, ud1, ue0, ue1, fa0, fa1, fb0, fb1)
    return g, arrs, flag_t


def _pick_rmacro(rows_per_core):
    for r in (512, 256, 128):
        if rows_per_core % r == 0:
            return r
    raise ValueError(f"rows_per_core {rows_per_core} not divisible by 128")


def _core_layouts(x_bf, sl, NT, R):
    """Per-core straight + transposed DRAM layouts from bf16 rows."""
    RC = R // P
    KE = E // P
    xc = x_bf[sl]
    xs = np.ascontiguousarray(
        xc.reshape(NT, RC, P, E).transpose(0, 2, 1, 3))
    xt = np.ascontiguousarray(
        xc.reshape(NT, R, KE, P).transpose(0, 3, 2, 1))
    return xs, xt


def prepare(inputs):
    """Compile (cached) + build per-core input maps.

    Returns (nc, in_maps, rows_per_core)."""
    g, arrs, flag_t = _prep_host(inputs)
    B = g["dna"].shape[0]
    rows_per_core = B // NCORES
    rmacro = _pick_rmacro(rows_per_core)
    key = ("v4", rows_per_core, rmacro, flag_t)
    if key not in _prog_cache:
        _prog_cache[key] = _build_program(rows_per_core, rmacro, flag_t)
    nc = _prog_cache[key]

    NT = rows_per_core // rmacro
    dna_bf = np.asarray(g["dna"], dtype=BF)
    mol_bf = np.asarray(g["mol"], dtype=BF)
    in_maps = []
    for i in range(NCORES):
        sl = slice(i * rows_per_core, (i + 1) * rows_per_core)
        xs0, xt0 = _core_layouts(dna_bf, sl, NT, rmacro)
        xs1, xt1 = _core_layouts(mol_bf, sl, NT, rmacro)
        im = {"xs_dna": xs0, "xt_dna": xt0, "xs_mol": xs1, "xt_mol": xt1}
        im.update(arrs)
        in_maps.append(im)
    return nc, in_maps, rows_per_core


def gather_out(res, rows_per_core):
    outs = []
    for r in res.results:
        o = r["out"]  # [NT, 2, P, RC, E]
        o = o.transpose(0, 3, 2, 1, 4).reshape(rows_per_core, 2 * E)
        outs.append(o)
    return np.concatenate(outs, axis=0).astype(np.float32)


def kernel(**inputs):
    nc, in_maps, rows_per_core = prepare(inputs)
    res = run_bass_kernel_spmd(nc, in_maps, list(range(NCORES)))
    return gather_out(res, rows_per_core)
